# revision 3
# baseline (speedup 1.0000x reference)
"""ContextSNN (2-layer LIF spiking net, T=50) on 8 Trainium2 NeuronCores.

Strategy:
  - Data-parallel: batch B=4096 sharded 512 per core; weights replicated.
  - fc1 is a SINGLE fp16 matmul pass per step with mean-centered operands:
    host ships xc = fp16(x - 0.5). Centering halves both the x-rounding
    residual and the |x| magnitude multiplying the W1-rounding residual,
    so one fp16 pass lands within the spike-count error tolerance
    (the LIF dynamics are mildly chaotic; rel err ~1.4e-2 vs the 2e-2
    gate, deterministic).
  - Layer-1 spikes are stored centered (sc = s - 0.5, exact in fp16);
    every resulting constant drift folds exactly (float64 on host) into
    per-neuron threshold/init constants:
      C1 = b1 + 0.5*sum(W1,in) - THR/2,  thr1 = THR - C1/(1-beta)
      C2 = b2 + 0.5*sum(W2,h),           thr2 = THR - C2/(1-beta)
  - fc2 = sc @ w2h single fp16 pass (4 small matmuls/step).
  - LIF state stays on-chip in [H, B] layout; each step is two
    scalar_tensor_tensor ops plus one fused compare-and-center
    ((u > thr) - 0.5) per 128-row block.
  - fc2 matmuls of step t-1 are slotted behind step t's first fc1 block
    so the PE never stalls on the DVE LIF chain.
"""
import sys
sys.path.insert(0, "/opt/trn_rl_repo")
import numpy as np
from contextlib import ExitStack

import concourse.bass as bass
import concourse.tile as tile
from concourse import bacc, mybir
from concourse.bass_utils import run_bass_kernel_spmd

T, B, IN, H, OUT = 50, 4096, 1500, 512, 45
INP = 1536          # IN padded to 12*128
NCORES = 8
BS = B // NCORES    # 512
BETA, THR = 0.9, 1.0
KT1 = INP // 128    # 12
MT1 = H // 128      # 4
KT2 = H // 128      # 4
FC2P = 1            # fc2 hi/lo passes
f16 = mybir.dt.float16
f32 = mybir.dt.float32
ALU = mybir.AluOpType

_NC_CACHE = {}


def _build():
    if "nc" in _NC_CACHE:
        return _NC_CACHE["nc"]
    nc = bacc.Bacc("TRN2", target_bir_lowering=False, debug=False, num_devices=NCORES)

    xc_d = nc.dram_tensor("xc", [T, INP, BS], f16, kind="ExternalInput").ap()
    w1h_d = nc.dram_tensor("w1h", [INP, H], f16, kind="ExternalInput").ap()
    w2h_d = nc.dram_tensor("w2h", [H, OUT], f16, kind="ExternalInput").ap()
    if FC2P == 2:
        w2l_d = nc.dram_tensor("w2l", [H, OUT], f16, kind="ExternalInput").ap()
    thr1_d = nc.dram_tensor("thr1", [128, MT1], f32, kind="ExternalInput").ap()
    u1i_d = nc.dram_tensor("u1i", [128, MT1], f32, kind="ExternalInput").ap()
    thr2_d = nc.dram_tensor("thr2", [OUT, 1], f32, kind="ExternalInput").ap()
    u2i_d = nc.dram_tensor("u2i", [OUT, 1], f32, kind="ExternalInput").ap()
    out_d = nc.dram_tensor("out", [OUT, BS], f32, kind="ExternalOutput").ap()

    with tile.TileContext(nc) as tc:
        with ExitStack() as ctx:
            wpool = ctx.enter_context(tc.tile_pool(name="w", bufs=1))
            xcpool = ctx.enter_context(tc.tile_pool(name="xc", bufs=48))
            state = ctx.enter_context(tc.tile_pool(name="state", bufs=1))
            spk1pool = ctx.enter_context(tc.tile_pool(name="spk1", bufs=2))
            spk2pool = ctx.enter_context(tc.tile_pool(name="spk2", bufs=2))
            t1pool = ctx.enter_context(tc.tile_pool(name="t1", bufs=8))
            t2pool = ctx.enter_context(tc.tile_pool(name="t2", bufs=2))
            ps1 = ctx.enter_context(tc.tile_pool(name="ps1", bufs=6, space="PSUM"))
            ps2 = ctx.enter_context(tc.tile_pool(name="ps2", bufs=2, space="PSUM"))

            # small state-constant DMAs go first on the gpsimd queue so the
            # DVE state inits (which the FIFO DVE queue head-of-line blocks
            # on) aren't stuck behind ~2MB of weight DMAs.
            thr1 = wpool.tile([128, MT1], f32, tag="thr1")
            u1i = wpool.tile([128, MT1], f32, tag="u1i")
            nc.gpsimd.dma_start(thr1[:], thr1_d[:])
            nc.gpsimd.dma_start(u1i[:], u1i_d[:])
            thr2 = wpool.tile([OUT, 1], f32, tag="thr2")
            u2i = wpool.tile([OUT, 1], f32, tag="u2i")
            nc.gpsimd.dma_start(thr2[:], thr2_d[:])
            nc.gpsimd.dma_start(u2i[:], u2i_d[:])

            # dummy matmuls on a zeroed tile, emitted before the state inits
            # so they start immediately: warm the PE HAM clock gate to
            # 2.4 GHz during the initial DMA wait (psum never read).
            warm = state.tile([128, BS], f16, tag="warm")
            nc.vector.memset(warm[:], 0.0)
            for _ in range(16):
                pw = ps1.tile([128, BS], f32, tag="p1")
                nc.tensor.matmul(pw[:], warm[:, 0:128], warm[:], start=True, stop=True)

            # one tile per k-chunk: the first matmul only waits on the k=0
            # DMA instead of all of them (tile-granular dependencies).
            w1h_t = []
            for k in range(KT1):
                wht = wpool.tile([128, H], f16, tag=f"w1h{k}")
                nc.gpsimd.dma_start(wht[:], w1h_d[k * 128:(k + 1) * 128, :])
                w1h_t.append(wht)
            w2h = wpool.tile([128, KT2 * OUT], f16, tag="w2h")
            for k in range(KT2):
                nc.gpsimd.dma_start(w2h[:, k * OUT:(k + 1) * OUT], w2h_d[k * 128:(k + 1) * 128, :])
            if FC2P == 2:
                w2l = wpool.tile([128, KT2 * OUT], f16, tag="w2l")
                for k in range(KT2):
                    nc.gpsimd.dma_start(w2l[:, k * OUT:(k + 1) * OUT], w2l_d[k * 128:(k + 1) * 128, :])
            zeros1 = state.tile([128, BS], f32, tag="zeros1")
            nc.vector.memset(zeros1[:], 0.0)
            u1 = state.tile([128, MT1 * BS], f32, tag="u1")
            for m in range(MT1):
                nc.vector.tensor_scalar(
                    u1[:, m * BS:(m + 1) * BS], zeros1[:], u1i[:, m:m + 1], None, ALU.add
                )
            zeros2 = state.tile([OUT, BS], f32, tag="zeros2")
            nc.vector.memset(zeros2[:], 0.0)
            u2 = state.tile([OUT, BS], f32, tag="u2")
            nc.vector.tensor_scalar(u2[:], zeros2[:], u2i[:, 0:1], None, ALU.add)
            acc = state.tile([OUT, BS], f32, tag="acc")
            nc.vector.memset(acc[:], 0.0)

            # layer-1 spikes stored centered: -0.5 = "no spike"
            spk1_prev = spk1pool.tile([128, MT1 * BS], f16)
            nc.vector.memset(spk1_prev[:], -0.5)
            spk2_prev = spk2pool.tile([OUT, BS], f32)
            nc.vector.memset(spk2_prev[:], 0.0)

            nmm1 = KT1
            nmm2 = KT2 * FC2P

            def emit_fc2_mms(spk1_t):
                p2 = ps2.tile([OUT, BS], f32)
                mm = 0
                for k in range(KT2):
                    ksl = slice(k * OUT, (k + 1) * OUT)
                    bsl = slice(k * BS, (k + 1) * BS)
                    nc.tensor.matmul(p2[:], w2h[:, ksl], spk1_t[:, bsl],
                                     start=(mm == 0), stop=(mm + 1 == nmm2))
                    mm += 1
                    if FC2P == 2:
                        nc.tensor.matmul(p2[:], w2l[:, ksl], spk1_t[:, bsl],
                                         start=False, stop=(mm + 1 == nmm2))
                        mm += 1
                return p2

            def emit_lif2(p2, spk2_p):
                t2 = t2pool.tile([OUT, BS], f32, tag="t2")
                nc.vector.scalar_tensor_tensor(
                    t2[:], spk2_p[:], -THR, p2[:], ALU.mult, ALU.add
                )
                nc.vector.scalar_tensor_tensor(
                    u2[:], u2[:], BETA, t2[:], ALU.mult, ALU.add
                )
                spk2_new = spk2pool.tile([OUT, BS], f32)
                nc.vector.tensor_scalar(
                    spk2_new[:], u2[:], thr2[:, 0:1], None, ALU.is_gt
                )
                nc.vector.tensor_add(acc[:], acc[:], spk2_new[:])
                return spk2_new

            for t in range(T):
                xc_t = []
                for k in range(KT1):
                    eng = nc.sync if (k % 2 == 0) else nc.scalar
                    xt = xcpool.tile([128, BS], f16, tag="xc")
                    eng.dma_start(xt[:], xc_d[t, k * 128:(k + 1) * 128, :])
                    xc_t.append(xt)

                spk1_new = spk1pool.tile([128, MT1 * BS], f16)

                for m in range(MT1):
                    sl = slice(m * 128, (m + 1) * 128)
                    p1 = ps1.tile([128, BS], f32)
                    for k in range(KT1):
                        nc.tensor.matmul(p1[:], w1h_t[k][:, sl], xc_t[k][:],
                                         start=(k == 0), stop=(k == nmm1 - 1))
                    if m == 0 and t > 0:
                        # previous step's fc2 matmuls slot in behind this
                        # step's first fc1 block: their spike inputs are
                        # ready, so PE never stalls on the DVE LIF chain.
                        p2_pending = emit_fc2_mms(spk1_prev)
                    msl = slice(m * BS, (m + 1) * BS)
                    t1 = t1pool.tile([128, BS], f32, tag="t1")
                    nc.vector.scalar_tensor_tensor(
                        t1[:], spk1_prev[:, msl], -THR, p1[:], ALU.mult, ALU.add
                    )
                    nc.vector.scalar_tensor_tensor(
                        u1[:, msl], u1[:, msl], BETA, t1[:], ALU.mult, ALU.add
                    )
                    # centered spike: (u > thr) - 0.5 fused in one DVE op
                    nc.vector.tensor_scalar(
                        spk1_new[:, msl], u1[:, msl], thr1[:, m:m + 1], 0.5,
                        ALU.is_gt, ALU.subtract
                    )
                if t > 0:
                    # lif2 DVE ops stay at the m-loop tail so the DVE stream
                    # never head-of-line blocks on the fc2 psum.
                    spk2_prev = emit_lif2(p2_pending, spk2_prev)
                spk1_prev = spk1_new

            spk2_prev = emit_lif2(emit_fc2_mms(spk1_prev), spk2_prev)

            nc.sync.dma_start(out_d[:], acc[:])

    nc.compile()
    _NC_CACHE["nc"] = nc
    return nc


def prep_in_maps(spike_seq, W1, b1, W2, b2):
    x = np.asarray(spike_seq, dtype=np.float32)
    W1 = np.asarray(W1, dtype=np.float64)
    b1 = np.asarray(b1, dtype=np.float64)
    W2 = np.asarray(W2, dtype=np.float64)
    b2 = np.asarray(b2, dtype=np.float64)

    W1T32 = np.zeros((INP, H), np.float32)
    W1T32[:IN] = W1.T.astype(np.float32)
    w1h = W1T32.astype(np.float16)
    W2T32 = W2.T.astype(np.float32).copy()
    w2h = W2T32.astype(np.float16)
    w2l = (W2T32 - w2h.astype(np.float32)).astype(np.float16)

    # exact constant folds (float64): x centering, centered spk1, bias
    C1 = b1 + 0.5 * W1.sum(axis=1) - 0.5 * THR
    C2 = b2 + 0.5 * W2.sum(axis=1)
    thr1 = (THR - C1 / (1.0 - BETA)).astype(np.float32).reshape(MT1, 128).T.copy()
    u1i = (-C1 / (1.0 - BETA)).astype(np.float32).reshape(MT1, 128).T.copy()
    thr2 = (THR - C2 / (1.0 - BETA)).astype(np.float32).reshape(OUT, 1)
    u2i = (-C2 / (1.0 - BETA)).astype(np.float32).reshape(OUT, 1)

    common = dict(w1h=w1h, w2h=w2h, thr1=thr1, u1i=u1i, thr2=thr2, u2i=u2i)
    if FC2P == 2:
        common["w2l"] = w2l

    xc_full = (x - 0.5).astype(np.float16)          # [T, B, IN]

    in_maps = []
    for c in range(NCORES):
        cs, ce = c * BS, (c + 1) * BS
        xc_c = np.zeros((T, INP, BS), np.float16)
        xc_c[:, :IN, :] = xc_full[:, cs:ce, :].transpose(0, 2, 1)
        m = dict(common)
        m["xc"] = xc_c
        in_maps.append(m)
    return in_maps


def gather_out(res):
    return np.concatenate(
        [res.results[c]["out"].T for c in range(NCORES)], axis=0
    ).astype(np.float32)


def kernel(spike_seq, W1, b1, W2, b2):
    nc = _build()
    in_maps = prep_in_maps(spike_seq, W1, b1, W2, b2)
    res = run_bass_kernel_spmd(nc, in_maps, core_ids=list(range(NCORES)))
    return gather_out(res)

